# revision 14
# baseline (speedup 1.0000x reference)
"""Transformer-XL CompressiveLayer on 8 TRN2 NeuronCores.

Sharding: core c = (batch b = c//2) x (head-half hh = c%2).
Each core handles one batch's full 896 tokens with 8 of 16 heads and
2048 of 4096 FFN inner channels.  One pairwise exchange (bf16) joins the
attention output halves before the post-LN; the FFN partial outputs are
summed on the host (plus ff2 bias).

Attention uses the exp-product form: P = exp(s*AC) * exp(s*BD_shifted).
exp(s*BD) is written per head as ONE strided DMA into a padded DRAM
buffer (row stride K+1, col 0 preset to 1.0 = exp(0)) and read back as
ONE DMA-transpose, which yields the rel-shifted exp(BDs)^T directly in
[j, i] layout.  S^T = exp(s*AC^T) (Act) * shifted term (DVE), so no
P-transpose is needed before PV.  The PV stationary matrix carries an
appended ones column, making row 64 of the PV output the softmax
denominator; a 1-partition reciprocal + broadcast matmul normalizes the
attention vectors after PV.
"""

import numpy as np
import ml_dtypes
from contextlib import ExitStack

import concourse.bass as bass
import concourse.tile as tile
from concourse import mybir, bacc
from concourse.bass_utils import run_bass_kernel_spmd

F32 = mybir.dt.float32
BF16 = mybir.dt.bfloat16
BF = ml_dtypes.bfloat16

D, H, DH, FF = 1024, 16, 64, 4096
S, B, M, CM = 512, 4, 256, 128
K = S + M + CM          # 896 tokens
NT = K // 128           # 7 token tiles
ND = D // 128           # 8 D chunks
HC = 8                  # heads per core
FC = HC * DH            # 512 features per core
NFC = FC // 128         # 4 feature chunks per core
FFC = FF // 2           # 2048 ffn channels per core
NFF = FFC // 128        # 16 ffn chunks per core
SCALE = 1.0 / np.sqrt(DH)
EPS = 1e-5
PVLAG = 2               # heads of pipeline depth between S^T and PV

_CACHED = {}


def _ln_tile(nc, x_t, eps_sb, stat_pool):
    """In: x_t [128,1024]. Returns (mean, rstd) [128,1] f32 tiles."""
    stats = stat_pool.tile([128, 2, 6], F32, tag="stats")
    for g in range(2):
        nc.vector.bn_stats(out=stats[:, g, :], in_=x_t[:, g * 512:(g + 1) * 512])
    mv = stat_pool.tile([128, 2], F32, tag="mv")
    nc.vector.bn_aggr(out=mv, in_=stats)
    rstd = stat_pool.tile([128, 1], F32, tag="rstd")
    nc.scalar.activation(out=rstd, in_=mv[:, 1:2],
                         func=mybir.ActivationFunctionType.Sqrt,
                         bias=eps_sb, scale=1.0)
    nc.vector.reciprocal(out=rstd, in_=rstd)
    return mv[:, 0:1], rstd


def build(debug=False, sim_mode=False):
    nc = bacc.Bacc(None)

    xb = nc.declare_dram_parameter("xb", [128, NT, D], BF16, isOutput=False)
    posT = nc.declare_dram_parameter("posT", [128, NT, ND, 128], BF16,
                                     isOutput=False)
    qw = nc.declare_dram_parameter("qw", [128, ND, FC], BF16, isOutput=False)
    kw = nc.declare_dram_parameter("kw", [128, ND, FC], BF16, isOutput=False)
    vw = nc.declare_dram_parameter("vw", [128, ND, FC], BF16, isOutput=False)
    rw = nc.declare_dram_parameter("rw", [128, ND, FC], BF16, isOutput=False)
    ow = nc.declare_dram_parameter("ow", [128, NFC, D], BF16, isOutput=False)
    ff1w = nc.declare_dram_parameter("ff1w", [128, ND, FFC], BF16, isOutput=False)
    ff2w = nc.declare_dram_parameter("ff2w", [128, NFF, D], BF16, isOutput=False)
    rwb = nc.declare_dram_parameter("rwb", [128, NFC], F32, isOutput=False)
    rrb = nc.declare_dram_parameter("rrb", [128, NFC], F32, isOutput=False)
    ff1b = nc.declare_dram_parameter("ff1b", [128, NFF], F32, isOutput=False)
    ln1w = nc.declare_dram_parameter("ln1w", [D], BF16, isOutput=False)
    ln1b = nc.declare_dram_parameter("ln1b", [D], BF16, isOutput=False)
    ln2w = nc.declare_dram_parameter("ln2w", [D], BF16, isOutput=False)
    ln2b = nc.declare_dram_parameter("ln2b", [D], BF16, isOutput=False)

    out = nc.declare_dram_parameter("out", [128, NT, D], F32, isOutput=True)
    if debug:
        dbg_w = nc.declare_dram_parameter("dbg_w", [128, NT, D], F32,
                                          isOutput=True)
        dbg_pt = nc.declare_dram_parameter("dbg_pt", [128, NT, K], F32,
                                           isOutput=True)
        dbg_avt = nc.declare_dram_parameter("dbg_avt", [128, NFC, K], F32,
                                            isOutput=True)
        dbg_ao = nc.declare_dram_parameter("dbg_ao", [128, NT, D], F32,
                                           isOutput=True)
        dbg_ares = nc.declare_dram_parameter("dbg_ares", [128, NT, D], F32,
                                             isOutput=True)

    pads = [nc.dram_tensor(f"pad{h}", [K * (K + 1)], BF16) for h in range(HC)]
    ao_in0 = nc.dram_tensor("ao_in0", [128, 4, D], BF16)
    ao_out0 = nc.dram_tensor("ao_out0", [128, 4, D], BF16)
    ao_in1 = nc.dram_tensor("ao_in1", [128, 3, D], BF16)
    ao_out1 = nc.dram_tensor("ao_out1", [128, 3, D], BF16)

    with tile.TileContext(nc) as tc, ExitStack() as ctx:
        consts = ctx.enter_context(tc.tile_pool(name="consts", bufs=1))
        stat_pool = ctx.enter_context(tc.tile_pool(name="stats", bufs=8))
        # PSUM: tag "s" 2 banks x2, tag "pv" 2 banks x2 -> 8 banks
        psS = ctx.enter_context(tc.tile_pool(name="psS", bufs=2, space="PSUM"))

        eps_sb = consts.tile([128, 1], F32)
        nc.vector.memset(eps_sb, EPS)
        ones_row = consts.tile([128, NT], BF16)
        nc.vector.memset(ones_row, 1.0)
        zero_row = consts.tile([128, NT], BF16)
        nc.vector.memset(zero_row, 0.0)
        ones64 = consts.tile([1, 64], BF16)
        nc.vector.memset(ones64, 1.0)
        rwb_sb = consts.tile([128, NFC], F32)
        nc.gpsimd.dma_start(out=rwb_sb, in_=rwb[:])
        rrb_sb = consts.tile([128, NFC], F32)
        nc.gpsimd.dma_start(out=rrb_sb, in_=rrb[:])
        ff1b_sb = consts.tile([128, NFF], F32)
        nc.gpsimd.dma_start(out=ff1b_sb, in_=ff1b[:])

        def bcast(name, param):
            t = consts.tile([128, D], BF16, tag=name)
            nc.gpsimd.dma_start(out=t, in_=bass.AP(tensor=param, offset=0,
                                                   ap=[[0, 128], [1, D]]))
            return t

        ln1w_bc = bcast("ln1w_bc", ln1w)
        ln1b_bc = bcast("ln1b_bc", ln1b)
        ln2w_bc = bcast("ln2w_bc", ln2w)
        ln2b_bc = bcast("ln2b_bc", ln2b)

        # rel-shift zero column: variant-A heads hold exp'd values (1.0 =
        # exp(0)); variant-B heads hold raw logits (0.0)
        for h, pad in enumerate(pads):
            nc.gpsimd.dma_start(
                out=bass.AP(tensor=pad, offset=0,
                            ap=[[K + 1, 128], [(K + 1) * 128, NT]]),
                in_=ones_row)

        # persists through P2/P3
        persist = ctx.enter_context(tc.tile_pool(name="persist", bufs=1))
        w_sb = persist.tile([128, NT, D], BF16)      # LN1 output (residual)
        avT = persist.tile([128, NFC, K], BF16)
        # attention tensors: live P0..end of P1 only
        attn_stack = ExitStack()
        attn = attn_stack.enter_context(tc.tile_pool(name="attn", bufs=1))
        q1T = attn.tile([128, NFC, K], BF16)         # (q + r_w_bias)^T
        q2T = attn.tile([128, NFC, K], BF16)         # (q + r_r_bias)^T
        kT = attn.tile([128, NFC, K], BF16)
        rT = attn.tile([128, NFC, K], BF16)
        v1 = attn.tile([128, NT, HC, DH + 1], BF16)  # v + ones column
        nc.vector.memset(v1[:, :, :, DH:DH + 1], 1.0)

        # ---------------- P0: loads, LN1, transpose, projections ---------
        with tc.tile_pool(name="p0w", bufs=1) as p0w, \
             tc.tile_pool(name="p0", bufs=3) as p0:
            xb_sb = p0w.tile([128, NT, D], BF16)
            nc.sync.dma_start(out=xb_sb, in_=xb[:])
            posT_sb = p0w.tile([128, NT, ND, 128], BF16)
            nc.scalar.dma_start(out=posT_sb, in_=posT[:])
            rw_sb = p0w.tile([128, ND, FC], BF16)
            nc.scalar.dma_start(out=rw_sb, in_=rw[:])
            vw_sb = p0w.tile([128, ND, FC], BF16)
            nc.scalar.dma_start(out=vw_sb, in_=vw[:])
            qw_sb = p0w.tile([128, ND, FC], BF16)
            nc.scalar.dma_start(out=qw_sb, in_=qw[:])
            kw_sb = p0w.tile([128, ND, FC], BF16)
            nc.scalar.dma_start(out=kw_sb, in_=kw[:])
            wT = p0w.tile([128, NT, ND, 128], BF16)

            for t in range(NT):
                x_t = xb_sb[:, t, :]
                mean, rstd = _ln_tile(nc, x_t, eps_sb, stat_pool)
                wn = p0.tile([128, D], BF16, tag="wn")
                nc.vector.tensor_scalar(out=wn, in0=x_t, scalar1=mean,
                                        scalar2=rstd,
                                        op0=mybir.AluOpType.subtract,
                                        op1=mybir.AluOpType.mult)
                wg = p0.tile([128, D], BF16, tag="wg")
                nc.vector.tensor_mul(out=wg, in0=wn, in1=ln1w_bc)
                nc.vector.tensor_add(out=w_sb[:, t, :], in0=wg, in1=ln1b_bc)
                nc.sync.dma_start_transpose(wT[:, t, :, :], w_sb[:, t, :])

            def projT(w_sb_, dst, bias1=None, dst2=None, bias2=None,
                      rhs_sb=None):
                # dst[., fc, i] = (w_sb_[:, :, fc-block].T @ w^T)[feat, tok]
                for fc in range(NFC):
                    ps = psS.tile([128, 1024], F32, tag="s")
                    for dc in range(ND):
                        st = dict(start=(dc == 0), stop=(dc == ND - 1))
                        lh = w_sb_[:, dc, fc * 128:(fc + 1) * 128]
                        r = rhs_sb if rhs_sb is not None else wT
                        nc.tensor.matmul(ps[:, 0:512], lh, r[:, 0:4, dc, :],
                                         **st)
                        nc.tensor.matmul(ps[:, 512:896], lh, r[:, 4:7, dc, :],
                                         **st)
                    if bias1 is not None:
                        nc.vector.tensor_scalar_add(
                            out=dst[:, fc, :], in0=ps[:, 0:K],
                            scalar1=bias1[:, fc:fc + 1])
                        nc.vector.tensor_scalar_add(
                            out=dst2[:, fc, :], in0=ps[:, 0:K],
                            scalar1=bias2[:, fc:fc + 1])
                    else:
                        nc.scalar.copy(out=dst[:, fc, :], in_=ps[:, 0:K])

            # r first: depends only on posT+rw loads, streams while LN runs
            projT(rw_sb, rT, rhs_sb=posT_sb)
            for t in range(NT):
                pv = psS.tile([128, 1024], F32, tag="s")
                for dc in range(ND):
                    nc.tensor.matmul(pv[:, 0:512], wT[:, t, dc, :],
                                     vw_sb[:, dc, :],
                                     start=(dc == 0), stop=(dc == ND - 1))
                nc.vector.tensor_copy(
                    out=v1[:, t, :, 0:DH],
                    in_=pv[:, 0:512].rearrange("p (h d) -> p h d", h=HC))
            projT(qw_sb, q1T, bias1=rwb_sb, dst2=q2T, bias2=rrb_sb)
            projT(kw_sb, kT)

        if debug:
            with tc.tile_pool(name="dbgw", bufs=2) as dbgp:
                for t in range(NT):
                    tmp = dbgp.tile([128, D], F32, tag="t")
                    nc.scalar.copy(out=tmp, in_=w_sb[:, t, :])
                    nc.sync.dma_start(out=dbg_w[:, t, :], in_=tmp)

        # ---------------- P1: attention ----------------
        bd_pool = attn_stack.enter_context(tc.tile_pool(name="bdp", bufs=2))
        sbdT_pool = attn_stack.enter_context(
            tc.tile_pool(name="sbdp", bufs=PVLAG + 1))
        pT_pool = attn_stack.enter_context(
            tc.tile_pool(name="ptp", bufs=PVLAG + 1))
        eac_pool = attn_stack.enter_context(tc.tile_pool(name="eacp", bufs=2))

        def bd_phase(h):
            fc, hp = h // 2, (h % 2) * DH
            bd_all = bd_pool.tile([128, NT, K], BF16, tag="bd")
            for it in range(NT):
                ps = psS.tile([128, 1024], F32, tag="s")
                lh = q2T[hp:hp + DH, fc, it * 128:(it + 1) * 128]
                nc.tensor.matmul(ps[:, 0:512], lh, rT[hp:hp + DH, fc, 0:512],
                                 start=True, stop=True)
                nc.tensor.matmul(ps[:, 512:896], lh,
                                 rT[hp:hp + DH, fc, 512:896],
                                 start=True, stop=True)
                nc.scalar.activation(out=bd_all[:, it, :], in_=ps[:, 0:K],
                                     func=mybir.ActivationFunctionType.Exp,
                                     scale=float(SCALE))
            nc.gpsimd.dma_start(
                out=bass.AP(tensor=pads[h], offset=1,
                            ap=[[K + 1, 128], [(K + 1) * 128, NT], [1, K]]),
                in_=bd_all)

        def st_phase(h):
            # sbdT[j, jt, i] = exp(s*BD_shifted)[i, jt*128+j] via DMA
            # transpose of the padded buffer read at flat offset K + K*i + j.
            fc, hp = h // 2, (h % 2) * DH
            sbdT = sbdT_pool.tile([128, NT, K], BF16, tag="sbdT")
            nc.sync.dma_start_transpose(
                sbdT, bass.AP(tensor=pads[h], offset=K,
                              ap=[[K, K], [1, K]]))
            pT_all = pT_pool.tile([128, NT, K], BF16, tag="pT")
            if True:
                eac_all = eac_pool.tile([128, NT, K], BF16, tag="eac")
                for jt in range(NT):
                    ps = psS.tile([128, 1024], F32, tag="s")
                    lh = kT[hp:hp + DH, fc, jt * 128:(jt + 1) * 128]
                    nc.tensor.matmul(ps[:, 0:512], lh,
                                     q1T[hp:hp + DH, fc, 0:512],
                                     start=True, stop=True)
                    nc.tensor.matmul(ps[:, 512:896], lh,
                                     q1T[hp:hp + DH, fc, 512:896],
                                     start=True, stop=True)
                    nc.scalar.activation(out=eac_all[:, jt, :],
                                         in_=ps[:, 0:K],
                                         func=mybir.ActivationFunctionType.Exp,
                                         scale=float(SCALE))
                for jt in range(NT):
                    nc.vector.tensor_mul(out=pT_all[:, jt, :],
                                         in0=eac_all[:, jt, :],
                                         in1=sbdT[:, jt, :])
            else:
                s_all = eac_pool.tile([128, NT, K], BF16, tag="eac")
                for jt in range(NT):
                    ps = psS.tile([128, 1024], F32, tag="s")
                    lh = kT[hp:hp + DH, fc, jt * 128:(jt + 1) * 128]
                    nc.tensor.matmul(ps[:, 0:512], lh,
                                     q1T[hp:hp + DH, fc, 0:512],
                                     start=True, stop=True)
                    nc.tensor.matmul(ps[:, 512:896], lh,
                                     q1T[hp:hp + DH, fc, 512:896],
                                     start=True, stop=True)
                    # psum frees immediately; raw S in bf16 SBUF
                    nc.vector.tensor_add(out=s_all[:, jt, :], in0=ps[:, 0:K],
                                         in1=sbdT[:, jt, :])
                for jt in range(NT):
                    nc.scalar.activation(out=pT_all[:, jt, :],
                                         in_=s_all[:, jt, :],
                                         func=mybir.ActivationFunctionType.Exp,
                                         scale=float(SCALE))
            return pT_all

        def pv_phase(h, pT_all):
            fc, hp = h // 2, (h % 2) * DH
            pvx = psS.tile([128, 1024], F32, tag="pv")
            for jt in range(NT):
                st = dict(start=(jt == 0), stop=(jt == NT - 1))
                lh = v1[:, jt, h, :]
                nc.tensor.matmul(pvx[0:DH + 1, 0:512], lh,
                                 pT_all[:, jt, 0:512], **st)
                nc.tensor.matmul(pvx[0:DH + 1, 512:896], lh,
                                 pT_all[:, jt, 512:896], **st)
            rden = stat_pool.tile([1, K], BF16, tag="rden")
            with nc.allow_low_precision(reason="softmax denom recip in bf16"):
                nc.vector.reciprocal(out=rden, in_=pvx[DH:DH + 1, 0:K])
            bc = psS.tile([128, 1024], F32, tag="s")
            nc.tensor.matmul(bc[0:DH, 0:512], ones64, rden[:, 0:512],
                             start=True, stop=True)
            nc.tensor.matmul(bc[0:DH, 512:896], ones64, rden[:, 512:896],
                             start=True, stop=True)
            bc_sb = eac_pool.tile([DH, K], BF16, tag="bc_sb")
            nc.scalar.copy(out=bc_sb, in_=bc[0:DH, 0:K])
            nc.vector.tensor_mul(out=avT[hp:hp + DH, fc, :],
                                 in0=pvx[0:DH, 0:K], in1=bc_sb)

        pending = {}
        for h in range(HC):
            bd_phase(h)
            if h >= PVLAG:
                pv_phase(h - PVLAG, pending.pop(h - PVLAG))
            pending[h] = st_phase(h)
        for h in range(HC - PVLAG, HC):
            pv_phase(h, pending.pop(h))

        if debug:
            with tc.tile_pool(name="dbgav", bufs=2) as dbgp:
                for fc in range(NFC):
                    tmp = dbgp.tile([128, K], F32, tag="t")
                    nc.scalar.copy(out=tmp, in_=avT[:, fc, :])
                    nc.sync.dma_start(out=dbg_avt[:, fc, :], in_=tmp)

        attn_stack.close()

        # ---------------- P2: o_proj, exchange, LN2, transpose ----------
        p23w = ctx.enter_context(tc.tile_pool(name="p23w", bufs=1))
        ow_sb = p23w.tile([128, NFC, D], BF16)
        nc.sync.dma_start(out=ow_sb, in_=ow[:])
        ff1w_sb = p23w.tile([128, ND, FFC], BF16)
        nc.gpsimd.dma_start(out=ff1w_sb, in_=ff1w[:])
        ff2w_sb = p23w.tile([128, NFF, D], BF16)
        nc.gpsimd.dma_start(out=ff2w_sb, in_=ff2w[:])
        hT = p23w.tile([128, NFF, K], BF16)
        ao_sb = p23w.tile([128, NT, D], BF16)
        aor_sb = p23w.tile([128, NT, D], BF16)
        ars_all = aor_sb  # LN2 output overwrites the exchanged ao in place
        arT = p23w.tile([128, NT, ND, 128], BF16)

        chunks = [(0, 4, ao_in0, ao_out0), (4, 3, ao_in1, ao_out1)]

        with tc.tile_pool(name="p2", bufs=2) as p2:
            for t0, nt_c, ain, aout in chunks:
                for it in range(t0, t0 + nt_c):
                    po = psS.tile([128, 1024], F32, tag="pv")
                    for fc in range(NFC):
                        st = dict(start=(fc == 0), stop=(fc == NFC - 1))
                        l = avT[:, fc, it * 128:(it + 1) * 128]
                        nc.tensor.matmul(po[:, 0:512], l, ow_sb[:, fc, 0:512],
                                         **st)
                        nc.tensor.matmul(po[:, 512:1024], l,
                                         ow_sb[:, fc, 512:1024], **st)
                    nc.scalar.copy(out=ao_sb[:, it, :], in_=po)
                nc.sync.dma_start(out=ain[:], in_=ao_sb[:, t0:t0 + nt_c, :])
                if sim_mode:
                    nc.gpsimd.dma_start(out=aout[:], in_=ain[:])
                else:
                    nc.gpsimd.collective_compute(
                        "AllReduce", mybir.AluOpType.add,
                        replica_groups=[[0, 1], [2, 3], [4, 5], [6, 7]],
                        ins=[ain[:]], outs=[aout[:]])
                nc.sync.dma_start(out=aor_sb[:, t0:t0 + nt_c, :], in_=aout[:])
                for it in range(t0, t0 + nt_c):
                    x2 = p2.tile([128, D], F32, tag="x2")
                    nc.vector.tensor_add(out=x2, in0=w_sb[:, it, :],
                                         in1=aor_sb[:, it, :])
                    mean, rstd = _ln_tile(nc, x2, eps_sb, stat_pool)
                    xn = p2.tile([128, D], BF16, tag="xn")
                    nc.vector.tensor_scalar(out=xn, in0=x2, scalar1=mean,
                                            scalar2=rstd,
                                            op0=mybir.AluOpType.subtract,
                                            op1=mybir.AluOpType.mult)
                    xg = p2.tile([128, D], BF16, tag="xg")
                    nc.vector.tensor_mul(out=xg, in0=xn, in1=ln2w_bc)
                    nc.vector.tensor_add(out=ars_all[:, it, :], in0=xg,
                                         in1=ln2b_bc)
                nc.sync.dma_start_transpose(arT[:, t0:t0 + nt_c, :, :],
                                            ars_all[:, t0:t0 + nt_c, :])

        if debug:
            with tc.tile_pool(name="dbga", bufs=2) as dbgp:
                for t in range(NT):
                    tmp = dbgp.tile([128, D], F32, tag="t")
                    nc.scalar.copy(out=tmp, in_=aor_sb[:, t, :])
                    nc.sync.dma_start(out=dbg_ao[:, t, :], in_=tmp)
                    tmp2 = dbgp.tile([128, D], F32, tag="t2")
                    nc.scalar.copy(out=tmp2, in_=ars_all[:, t, :])
                    nc.sync.dma_start(out=dbg_ares[:, t, :], in_=tmp2)

        # ---------------- P3: FFN ----------------
        with tc.tile_pool(name="p3", bufs=3) as p3:
            for half, (i0, n, sl) in enumerate(
                    [(0, 512, (0, 4)), (512, 384, (4, 7))]):
                for ffc in range(NFF):
                    ph = psS.tile([128, 1024], F32, tag="s")
                    for dc in range(ND):
                        st = dict(start=(dc == 0), stop=(dc == ND - 1))
                        lh = ff1w_sb[:, dc, ffc * 128:(ffc + 1) * 128]
                        nc.tensor.matmul(ph[:, 0:n], lh,
                                         arT[:, sl[0]:sl[1], dc, :], **st)
                    nc.scalar.activation(
                        out=hT[:, ffc, i0:i0 + n], in_=ph[:, 0:n],
                        func=mybir.ActivationFunctionType.Relu,
                        bias=ff1b_sb[:, ffc:ffc + 1], scale=1.0)
            for it in range(NT):
                po = psS.tile([128, 1024], F32, tag="pv")
                for ffc in range(NFF):
                    st = dict(start=(ffc == 0), stop=(ffc == NFF - 1))
                    l = hT[:, ffc, it * 128:(it + 1) * 128]
                    nc.tensor.matmul(po[:, 0:512], l, ff2w_sb[:, ffc, 0:512],
                                     **st)
                    nc.tensor.matmul(po[:, 512:1024], l,
                                     ff2w_sb[:, ffc, 512:1024], **st)
                ot = p3.tile([128, D], F32, tag="ot")
                nc.vector.tensor_copy(out=ot, in_=po)
                nc.sync.dma_start(out=out[:, it, :], in_=ot)

    nc.finalize()
    return nc


def prep_inputs(inputs):
    """Full inputs -> list of 8 per-core input maps."""
    x_nat = np.concatenate([inputs["input_ids"], inputs["mem"],
                            inputs["c_mem"]], axis=0)  # [K,B,D] f32
    # posT[p, t, dc, l] = pos[t*128 + l, dc*128 + p]
    posn = inputs["positional_embedding"].astype(BF)  # [K, D]
    posT_t = np.ascontiguousarray(
        posn.reshape(NT, 128, ND, 128).transpose(3, 0, 2, 1))

    qkv = inputs["qkv_w"]
    maps = []
    for c in range(8):
        b, hh = c // 2, c % 2
        Fs = slice(hh * FC, (hh + 1) * FC)
        FFs = slice(hh * FFC, (hh + 1) * FFC)

        def wchunk(wmat):  # [D, FC] -> [128, ND, FC] bf16
            return np.ascontiguousarray(
                wmat.astype(BF).reshape(ND, 128, -1).transpose(1, 0, 2))

        m = {
            "xb": np.ascontiguousarray(
                x_nat[:, b, :].astype(BF).reshape(NT, 128, D)
                .transpose(1, 0, 2)),
            "posT": posT_t,
            "qw": wchunk(qkv[:, 0 * H * DH:1 * H * DH][:, Fs]),
            "kw": wchunk(qkv[:, 1 * H * DH:2 * H * DH][:, Fs]),
            "vw": wchunk(qkv[:, 2 * H * DH:3 * H * DH][:, Fs]),
            "rw": wchunk(inputs["r_w"][:, Fs]),
            "ow": np.ascontiguousarray(
                inputs["o_w"][Fs, :].astype(BF)
                .reshape(NFC, 128, D).transpose(1, 0, 2)),
            "ff1w": wchunk(inputs["ff1_w"][:, FFs]),
            "ff2w": np.ascontiguousarray(
                inputs["ff2_w"][FFs, :].astype(BF)
                .reshape(NFF, 128, D).transpose(1, 0, 2)),
            "rwb": np.ascontiguousarray(
                inputs["r_w_bias"][hh * HC:(hh + 1) * HC].reshape(-1)
                .reshape(NFC, 128).T.astype(np.float32)),
            "rrb": np.ascontiguousarray(
                inputs["r_r_bias"][hh * HC:(hh + 1) * HC].reshape(-1)
                .reshape(NFC, 128).T.astype(np.float32)),
            "ff1b": np.ascontiguousarray(
                inputs["ff1_b"][FFs].reshape(NFF, 128).T.astype(np.float32)),
            "ln1w": np.asarray(inputs["ln1_w"]).astype(BF),
            "ln1b": np.asarray(inputs["ln1_b"]).astype(BF),
            "ln2w": np.asarray(inputs["ln2_w"]).astype(BF),
            "ln2b": np.asarray(inputs["ln2_b"]).astype(BF),
        }
        maps.append(m)
    return maps


class PjrtRunner:
    """Persistent jitted SPMD executor for a prebuilt Bass module."""

    def __init__(self, nc, n_cores=8):
        import jax
        from jax.sharding import Mesh, PartitionSpec
        from jax.experimental.shard_map import shard_map
        from concourse import mybir as _mybir
        from concourse.bass2jax import (_bass_exec_p, install_neuronx_cc_hook,
                                        partition_id_tensor)
        install_neuronx_cc_hook()
        self.jax = jax
        self.n_cores = n_cores
        in_names, out_names, out_avals = [], [], []
        partition_name = (nc.partition_id_tensor.name
                          if nc.partition_id_tensor else None)
        for alloc in nc.m.functions[0].allocations:
            if not isinstance(alloc, _mybir.MemoryLocationSet):
                continue
            name = alloc.memorylocations[0].name
            if alloc.kind == "ExternalInput":
                if name != partition_name:
                    in_names.append(name)
            elif alloc.kind == "ExternalOutput":
                out_names.append(name)
                out_avals.append(jax.core.ShapedArray(
                    tuple(alloc.tensor_shape), _mybir.dt.np(alloc.dtype)))
        self.in_names, self.out_names, self.out_avals = \
            in_names, out_names, out_avals

        def _body(*args):
            operands = list(args)
            if partition_name is not None:
                operands.append(partition_id_tensor())
            all_in = in_names + out_names
            if partition_name is not None:
                all_in = all_in + [partition_name]
            return tuple(_bass_exec_p.bind(
                *operands,
                out_avals=tuple(out_avals),
                in_names=tuple(all_in),
                out_names=tuple(out_names),
                lowering_input_output_aliases=(),
                sim_require_finite=True,
                sim_require_nnan=True,
                nc=nc,
            ))

        devices = jax.devices()[:n_cores]
        self.mesh = Mesh(np.asarray(devices), ("core",))
        nin = len(in_names) + len(out_names)
        self.fn = jax.jit(shard_map(
            _body, mesh=self.mesh,
            in_specs=(PartitionSpec("core"),) * nin,
            out_specs=(PartitionSpec("core"),) * len(out_names),
            check_rep=False))

    def pack(self, maps):
        arrs = [self.jax.device_put(
                    np.concatenate([np.asarray(maps[c][n])
                                    for c in range(self.n_cores)], axis=0))
                for n in self.in_names]
        arrs += [self.jax.device_put(
                    np.zeros((self.n_cores * a.shape[0], *a.shape[1:]),
                             a.dtype))
                 for a in self.out_avals]
        return arrs

    def __call__(self, packed):
        return self.fn(*packed)

    def unpack(self, outs):
        res = []
        for c in range(self.n_cores):
            res.append({
                n: np.asarray(outs[i]).reshape(
                    self.n_cores, *self.out_avals[i].shape)[c]
                for i, n in enumerate(self.out_names)})
        return res


def get_runner(debug=False, sim_mode=False):
    key = (bool(debug), bool(sim_mode))
    if key not in _CACHED:
        nc = build(debug=debug, sim_mode=sim_mode)
        _CACHED[key] = PjrtRunner(nc, 8)
    return _CACHED[key]


def _unpack_out(arr):
    # [128, NT, D] -> [K, D]
    return np.asarray(arr, np.float32).transpose(1, 0, 2).reshape(K, D)


def _assemble(inputs, results):
    ff2b = np.asarray(inputs["ff2_b"], np.float32)
    out = np.zeros((K, B, D), np.float32)
    for b in range(B):
        out[:, b, :] = (_unpack_out(results[2 * b]["out"])
                        + _unpack_out(results[2 * b + 1]["out"])
                        + ff2b[None, :])
    return out


def run(inputs, trace=False, debug=False):
    runner = get_runner(debug=debug)
    maps = prep_inputs(inputs)
    packed = runner.pack(maps)
    outs = runner(packed)
    results = runner.unpack(outs)

    class R:
        pass
    res = R()
    res.results = results
    res.exec_time_ns = None
    return _assemble(inputs, results), res


def kernel(**inputs):
    inputs = {k: np.asarray(v) for k, v in inputs.items()}
    out, _ = run(inputs, trace=False, debug=False)
    return out
